# revision 6
# baseline (speedup 1.0000x reference)
"""Trainium2 Bass kernel for a 2-layer GCN encoder (N=100000, E=1600000, 128->128->64).

Strategy (8 NeuronCores, SPMD):
  out = A_hat @ relu(A_hat @ X @ W1 + b1) @ W2 + b2,  A_hat = D^-1/2 (A+I) D^-1/2

  - Destination nodes are bin-packed into 784 degree-balanced blocks of <=128
    dests (LPT), 98 blocks per core; edges live with their destination block.
  - Per-edge source rows are fetched with dma_gather (SWDGE int16 gather).
    int16 indices reach only 32768 rows, so edges are split into 4 source
    "buckets": layer 1 buckets by src%4 against a strided view of x
    (row stride 4 rows, base offset src%4), layer 2 buckets by position
    range in the all-gathered h2 table. Each (block, bucket) cell is padded
    to a uniform P_CELL chunks of 128 edges so one static program serves
    all cores; gather calls cover (group of G_BLK blocks) x (bucket).
  - Per chunk: build a norm-scaled one-hot [edge x dest] on the Vector
    engine (tensor_scalar: (iota == d_local) * norm) and matmul-accumulate
    gathered^T @ onehot into the block's PSUM accumulator R[feat, dest].
  - Layer 1 block tail: t1T = W1^T @ R1T (PE), h1T = relu(t1T + b1) (ACT),
    h2 = h1T^T @ W2 (PE) -> per-block h2 rows; AllGather h2 shards into a
    replicated 100352 x 64 table; layer 2 mirrors with + b2 (DVE) and a
    PE transpose for the output layout.
  - Host un-permutes block layout back to node order.
"""

import math

import numpy as np

N = 100000
E = 1600000
IN_F = 128
HID = 128
OUT_F = 64
NCORES = 8
P = 128
BLOCKS_PER_CORE = 98
NBLOCKS = NCORES * BLOCKS_PER_CORE  # 784
ROWS_PER_CORE = BLOCKS_PER_CORE * P  # 12544
G_BLK = 4       # blocks per gather-call group
NBUCKET = 4
L2_BUCKET_ROWS = 25088  # NCORES*ROWS_PER_CORE / 4, < 32768

_BUILD_CACHE = {}


# ----------------------------------------------------------------------------
# Host-side graph preprocessing
# ----------------------------------------------------------------------------

def _assign_blocks(deg):
    """LPT bin-packing of nodes into NBLOCKS blocks of <=128 nodes each,
    balancing per-block edge (degree) sums. Returns block_of, slot_of."""
    import heapq

    order = np.argsort(-deg, kind="stable")
    heap = [(0, 0, b) for b in range(NBLOCKS)]
    heapq.heapify(heap)
    block_of = np.empty(N, np.int64)
    slot_of = np.empty(N, np.int64)
    for node in order:
        load, cnt, b = heapq.heappop(heap)
        block_of[node] = b
        slot_of[node] = cnt
        cnt += 1
        load += int(deg[node])
        if cnt < P:
            heapq.heappush(heap, (load, cnt, b))
    return block_of, slot_of


def _groups():
    groups = []
    b0 = 0
    while b0 < BLOCKS_PER_CORE:
        nb = min(G_BLK, BLOCKS_PER_CORE - b0)
        groups.append((b0, nb))
        b0 += nb
    return groups


def _build_stream(bb_local, bucket, dloc, nrmv, idx16, p_cell):
    """Lay out one core's edges into the call-major padded stream.

    bb_local: [M] core-local dest block of each edge (0..97)
    bucket:   [M] bucket 0..3
    dloc:     [M] slot of dest within block (float)
    nrmv:     [M] edge norm (float)
    idx16:    [M] int16 gather index
    Returns (idx_stream int16 [S], dloc_stream f32 [S], nrm_stream f32 [S],
             ncols) with S = total padded slots; stream is ordered by
    (group, bucket, block-in-group, rank-in-cell) with each cell padded to
    p_cell*128 slots.
    """
    cell_cap = p_cell * P
    # cell id in stream order: group-major, then bucket, then block-in-group
    g = bb_local // G_BLK
    bl = bb_local % G_BLK
    # number of blocks in this edge's group (last group may be short)
    nb_in_group = np.minimum(BLOCKS_PER_CORE - g * G_BLK, G_BLK)
    # base slot of each group: full groups before it
    group_base = g * (G_BLK * NBUCKET * cell_cap)
    cell_in_group = bucket * nb_in_group + bl
    cell_base = group_base + cell_in_group * cell_cap

    # rank within cell
    key = bb_local * NBUCKET + bucket
    order = np.argsort(key, kind="stable")
    key_sorted = key[order]
    counts = np.bincount(key_sorted, minlength=BLOCKS_PER_CORE * NBUCKET)
    starts = np.zeros_like(counts)
    starts[1:] = np.cumsum(counts)[:-1]
    rank_sorted = np.arange(order.size, dtype=np.int64) - starts[key_sorted]
    rank = np.empty(order.size, dtype=np.int64)
    rank[order] = rank_sorted

    assert counts.max() <= cell_cap, (counts.max(), cell_cap)
    pos = cell_base + rank

    total = 0
    for _, nb in _groups():
        total += nb * NBUCKET * cell_cap
    idx_stream = np.zeros(total, np.int16)
    dloc_stream = np.zeros(total, np.float32)
    nrm_stream = np.zeros(total, np.float32)
    idx_stream[pos] = idx16
    dloc_stream[pos] = dloc
    nrm_stream[pos] = nrmv
    return idx_stream, dloc_stream, nrm_stream, total // P


def _pack_gidx(idx_stream):
    """int16 stream -> dma_gather SBUF layout [128, S/16], wrapped in 16
    partitions and replicated 8x."""
    m = idx_stream.reshape(-1, 16).T  # [16, S/16]
    return np.ascontiguousarray(np.tile(m, (8, 1)))


def _prep(x, edge_index, W1, b1, W2, b2):
    x = np.ascontiguousarray(np.asarray(x, dtype=np.float32))
    ei = np.asarray(edge_index, dtype=np.int64)
    row = np.concatenate([ei[0], np.arange(N, dtype=np.int64)])
    col = np.concatenate([ei[1], np.arange(N, dtype=np.int64)])

    degi = np.bincount(col, minlength=N)
    dinv = 1.0 / np.sqrt(degi.astype(np.float64))
    norm = (dinv[row] * dinv[col]).astype(np.float32)

    block_of, slot_of = _assign_blocks(degi)
    perm_pos = (block_of // BLOCKS_PER_CORE) * ROWS_PER_CORE + (
        block_of % BLOCKS_PER_CORE
    ) * P + slot_of

    core_of_edge = block_of[col] // BLOCKS_PER_CORE
    bb_local = block_of[col] % BLOCKS_PER_CORE
    dloc_all = slot_of[col].astype(np.float32)

    # buckets + int16 indices
    b1k = (row % NBUCKET).astype(np.int64)
    i1 = (row // NBUCKET).astype(np.int16)
    cpos = perm_pos[row]
    b2k = cpos // L2_BUCKET_ROWS
    i2 = (cpos - b2k * L2_BUCKET_ROWS).astype(np.int16)

    # uniform P_CELL per layer (max cell count over all cores)
    def cell_max(bucket):
        key = (core_of_edge * BLOCKS_PER_CORE + bb_local) * NBUCKET + bucket
        return np.bincount(key, minlength=NBLOCKS * NBUCKET).max()

    p1 = int(math.ceil(cell_max(b1k) / P))
    p2 = int(math.ceil(cell_max(b2k) / P))

    per_core = []
    for s in range(NCORES):
        m = core_of_edge == s
        s1_idx, s1_d, s1_n, nch1 = _build_stream(
            bb_local[m], b1k[m], dloc_all[m], norm[m], i1[m], p1
        )
        s2_idx, s2_d, s2_n, nch2 = _build_stream(
            bb_local[m], b2k[m], dloc_all[m], norm[m], i2[m], p2
        )
        per_core.append(
            {
                "gidx1": _pack_gidx(s1_idx),
                "gidx2": _pack_gidx(s2_idx),
                "dloc1": np.ascontiguousarray(s1_d.reshape(-1, P).T),
                "nrm1": np.ascontiguousarray(s1_n.reshape(-1, P).T),
                "dloc2": np.ascontiguousarray(s2_d.reshape(-1, P).T),
                "nrm2": np.ascontiguousarray(s2_n.reshape(-1, P).T),
            }
        )

    consts = {
        "x_full": x,
        "w1": np.ascontiguousarray(np.asarray(W1, dtype=np.float32)),
        "w2": np.ascontiguousarray(np.asarray(W2, dtype=np.float32)),
        "b1": np.ascontiguousarray(np.asarray(b1, np.float32).reshape(HID, 1)),
        "b2": np.ascontiguousarray(np.asarray(b2, np.float32).reshape(OUT_F, 1)),
        "iota": np.ascontiguousarray(np.tile(np.arange(P, dtype=np.float32), (P, 1))),
        "ident": np.eye(P, dtype=np.float32),
    }
    return (p1, p2), per_core, consts, perm_pos


# ----------------------------------------------------------------------------
# Bass program
# ----------------------------------------------------------------------------

def _build(p_cells):
    if p_cells in _BUILD_CACHE:
        return _BUILD_CACHE[p_cells]

    import concourse.bass as bass
    import concourse.bacc as bacc
    import concourse.mybir as mybir
    import concourse.tile as tile

    p1, p2 = p_cells
    f32 = mybir.dt.float32
    i16 = mybir.dt.int16
    groups = _groups()
    nch1 = sum(nb * NBUCKET * p1 for _, nb in groups)
    nch2 = sum(nb * NBUCKET * p2 for _, nb in groups)
    # x viewed as [N/4, 4, IN_F]: bucket k = x4[:, k, :]
    n4 = N // NBUCKET

    nc = bacc.Bacc(
        "TRN2", target_bir_lowering=False, debug=False, num_devices=NCORES
    )
    x_full = nc.dram_tensor("x_full", [N, IN_F], f32, kind="ExternalInput")
    w1 = nc.dram_tensor("w1", [IN_F, HID], f32, kind="ExternalInput")
    w2 = nc.dram_tensor("w2", [HID, OUT_F], f32, kind="ExternalInput")
    b1 = nc.dram_tensor("b1", [HID, 1], f32, kind="ExternalInput")
    b2 = nc.dram_tensor("b2", [OUT_F, 1], f32, kind="ExternalInput")
    iota = nc.dram_tensor("iota", [P, P], f32, kind="ExternalInput")
    ident = nc.dram_tensor("ident", [P, P], f32, kind="ExternalInput")
    gidx1 = nc.dram_tensor("gidx1", [P, nch1 * P // 16], i16, kind="ExternalInput")
    gidx2 = nc.dram_tensor("gidx2", [P, nch2 * P // 16], i16, kind="ExternalInput")
    dloc1 = nc.dram_tensor("dloc1", [P, nch1], f32, kind="ExternalInput")
    nrm1 = nc.dram_tensor("nrm1", [P, nch1], f32, kind="ExternalInput")
    dloc2 = nc.dram_tensor("dloc2", [P, nch2], f32, kind="ExternalInput")
    nrm2 = nc.dram_tensor("nrm2", [P, nch2], f32, kind="ExternalInput")
    out_local = nc.dram_tensor(
        "out_local", [ROWS_PER_CORE, OUT_F], f32, kind="ExternalOutput"
    )

    relu = mybir.ActivationFunctionType.Relu
    copyf = mybir.ActivationFunctionType.Copy
    is_eq = mybir.AluOpType.is_equal
    mult = mybir.AluOpType.mult
    add = mybir.AluOpType.add

    x4 = x_full[:].rearrange("(a b) f -> a b f", b=NBUCKET)

    with tile.TileContext(nc) as tc:
        with (
            tc.tile_pool(name="consts", bufs=1) as cp,
            tc.tile_pool(name="gat", bufs=2) as gat,
            tc.tile_pool(name="idxp", bufs=2) as idxp,
            tc.tile_pool(name="dnp", bufs=2) as dnp,
            tc.tile_pool(name="sp", bufs=6) as sp,
            tc.tile_pool(name="blk", bufs=3) as blk,
            tc.tile_pool(name="psacc", bufs=G_BLK, space="PSUM") as psacc,
            tc.tile_pool(name="psmid", bufs=2, space="PSUM") as psmid,
            tc.tile_pool(name="psout", bufs=2, space="PSUM") as psout,
            tc.tile_pool(name="dram", bufs=1, space="DRAM") as dram,
        ):
            w1t = cp.tile([IN_F, HID], f32)
            w2t = cp.tile([HID, OUT_F], f32)
            b1t = cp.tile([HID, 1], f32)
            b2t = cp.tile([OUT_F, 1], f32)
            iotat = cp.tile([P, P], f32)
            identt = cp.tile([P, P], f32)
            nc.sync.dma_start(w1t[:], w1[:])
            nc.sync.dma_start(w2t[:], w2[:])
            nc.sync.dma_start(b1t[:], b1[:])
            nc.sync.dma_start(b2t[:], b2[:])
            nc.sync.dma_start(iotat[:], iota[:])
            nc.sync.dma_start(identt[:], ident[:])

            h2_local = dram.tile([ROWS_PER_CORE, OUT_F], f32, tag="h2l")
            h2_full = dram.tile(
                [NCORES * ROWS_PER_CORE, OUT_F], f32, tag="h2f",
                addr_space="Shared",
            )

            def layer(
                feat, p_cell, gidx, dlocT, nrmT, gather_src_fn, tail_fn
            ):
                chunk_base = 0
                for b0, nb in groups:
                    ncell_ch = p_cell  # chunks per cell
                    call_ch = nb * ncell_ch  # chunks per gather call
                    gts = []
                    for k in range(NBUCKET):
                        nidx = call_ch * P
                        gt = gat.tile([P, call_ch * feat], f32, tag=f"g{k}")
                        it = idxp.tile([P, nidx // 16], i16, tag=f"i{k}")
                        c0 = chunk_base + k * call_ch
                        nc.sync.dma_start(
                            it[:], gidx[:, c0 * P // 16 : (c0 + call_ch) * P // 16]
                        )
                        nc.gpsimd.dma_gather(
                            out_ap=gt[:].rearrange("p (c e) -> p c e", e=feat),
                            in_ap=gather_src_fn(k),
                            idxs_ap=it[:],
                            num_idxs=nidx,
                            num_idxs_reg=nidx,
                            elem_size=feat,
                            elem_step=(512 if feat == IN_F else None),
                            single_packet=False,
                        )
                        gts.append(gt)
                    dt = dnp.tile([P, NBUCKET * call_ch], f32, tag="d")
                    nt = dnp.tile([P, NBUCKET * call_ch], f32, tag="n")
                    nc.sync.dma_start(
                        dt[:], dlocT[:, chunk_base : chunk_base + NBUCKET * call_ch]
                    )
                    nc.sync.dma_start(
                        nt[:], nrmT[:, chunk_base : chunk_base + NBUCKET * call_ch]
                    )

                    accs = []
                    for _bl in range(nb):
                        acc_t = psacc.tile([feat, P], f32, tag="acc")
                        accs.append(acc_t)
                    for k in range(NBUCKET):
                        gt = gts[k]
                        for bl in range(nb):
                            for j in range(ncell_ch):
                                c = bl * ncell_ch + j  # col within call
                                cg = k * call_ch + c  # col within dt/nt slice
                                st = sp.tile([P, P], f32, tag="s")
                                nc.vector.tensor_scalar(
                                    out=st[:],
                                    in0=iotat[:],
                                    scalar1=dt[:, cg : cg + 1],
                                    scalar2=nt[:, cg : cg + 1],
                                    op0=is_eq,
                                    op1=mult,
                                )
                                nc.tensor.matmul(
                                    accs[bl][:],
                                    lhsT=gt[:, c * feat : (c + 1) * feat],
                                    rhs=st[:],
                                    start=(k == 0 and j == 0),
                                    stop=(k == NBUCKET - 1 and j == ncell_ch - 1),
                                )
                    for bl in range(nb):
                        tail_fn(b0 + bl, accs[bl])
                    chunk_base += NBUCKET * call_ch

            # ---------------- Layer 1 ----------------
            def l1_tail(bb, acc):
                r1 = blk.tile([IN_F, P], f32, tag="r1")
                nc.scalar.activation(r1[:], acc[:], copyf)
                t1 = psmid.tile([HID, P], f32, tag="t1")
                nc.tensor.matmul(t1[:], lhsT=w1t[:], rhs=r1[:], start=True, stop=True)
                h1 = blk.tile([HID, P], f32, tag="h1")
                nc.scalar.activation(h1[:], t1[:], relu, bias=b1t[:, :1])
                h2p = psout.tile([P, OUT_F], f32, tag="h2p")
                nc.tensor.matmul(h2p[:], lhsT=h1[:], rhs=w2t[:], start=True, stop=True)
                h2s = blk.tile([P, OUT_F], f32, tag="h2s")
                nc.scalar.activation(h2s[:], h2p[:], copyf)
                nc.sync.dma_start(h2_local[bb * P : (bb + 1) * P, :], h2s[:])

            layer(
                IN_F, p1, gidx1, dloc1, nrm1,
                lambda k: x4[:, k, :], l1_tail,
            )

            # ---------------- AllGather ----------------
            nc.gpsimd.collective_compute(
                "AllGather",
                mybir.AluOpType.bypass,
                replica_groups=[list(range(NCORES))],
                ins=[h2_local.opt()],
                outs=[h2_full.opt()],
            )

            # ---------------- Layer 2 ----------------
            def l2_tail(bb, acc):
                r2 = blk.tile([OUT_F, P], f32, tag="r2")
                nc.vector.tensor_scalar(
                    out=r2[:], in0=acc[:], scalar1=b2t[:, :1], scalar2=None, op0=add
                )
                op = psout.tile([P, OUT_F], f32, tag="h2p")
                nc.tensor.transpose(op[:], r2[:], identt[:OUT_F, :OUT_F])
                os_ = blk.tile([P, OUT_F], f32, tag="h2s")
                nc.scalar.activation(os_[:], op[:], copyf)
                nc.sync.dma_start(out_local[bb * P : (bb + 1) * P, :], os_[:])

            layer(
                OUT_F, p2, gidx2, dloc2, nrm2,
                lambda k: h2_full[
                    k * L2_BUCKET_ROWS : (k + 1) * L2_BUCKET_ROWS, :
                ], l2_tail,
            )

    nc.compile()
    _BUILD_CACHE[p_cells] = nc
    return nc


# ----------------------------------------------------------------------------
# Entry point
# ----------------------------------------------------------------------------

def _run(inputs, trace=False):
    from concourse.bass_utils import run_bass_kernel_spmd

    p_cells, per_core, consts, perm_pos = _prep(
        inputs["x"], inputs["edge_index"], inputs["W1"], inputs["b1"],
        inputs["W2"], inputs["b2"],
    )
    nc = _build(p_cells)
    in_maps = [{**consts, **per_core[s]} for s in range(NCORES)]
    res = run_bass_kernel_spmd(
        nc, in_maps, core_ids=list(range(NCORES)), trace=trace
    )
    all_out = np.concatenate(
        [res.results[s]["out_local"] for s in range(NCORES)], axis=0
    )
    out = np.ascontiguousarray(all_out[perm_pos])
    return out, res


def kernel(**inputs) -> np.ndarray:
    out, _ = _run(inputs, trace=False)
    return out
